# revision 13
# baseline (speedup 1.0000x reference)
"""Trainium2 Bass kernel for nn_DeconvDft2dLayer.

Math reduction: w is [1, 8], so the padded filter hm1 occupies only row 0 of
the [H, W] grid. Hence fft2(hm1)[k, l] is independent of the row frequency k,
and the combined inverse-filter spectrum gmf[k, l] collapses to a real 1D
spectrum g1d[l] = |W1(l)|^-4 along W only (W1 = length-W FFT of the taps).
The H-axis FFT cancels with its inverse, so the whole layer is a per-row
circular convolution:

    y[b, h, :] = ifft(fft(x[b, h, :]) * g1d)  =  x[b, h, :] @ K

with K the real symmetric [W, W] circulant of ker = ifft(g1d), computed on
host from the 8 taps and replicated to all 8 cores; x is sharded over batch
(4 images per core).

Performance shape (per core): everything rides bf16 (absmax rel err ~4e-3
vs the 2e-2 gate; inputs are white noise so quantization error stays white
through the filter). IO drops to 2 MiB in + 2 MiB out + 0.3 MiB filter.
The circulant kernel decays fast enough that the far 128-block band
(|blockrow - blockcol| == 2 mod 4) contributes below bf16 noise, so each
128-row output chunk needs only 6 matmuls covering 1536 output columns
(vs 4x512 = 2048 for the full circulant): per block row j the kept columns
are the three blocks j-1, j, j+1 (mod 4), split at the wraparound.
PSUM start=True clears the whole bank's has_written bits, so partial-width
accumulation windows compose correctly.

Extras: dummy matmuls on a zeroed scratch tile warm the PE's HAM clock gate
(1.2 -> 2.4 GHz) during the initial DMA latency; x is re-laid-out on host so
every load group is one contiguous-per-partition DMA; K ships pre-rotated
from host (no on-device circulant build); the last chunk's PSUM->SBUF copy
is split across ScalarE+VectorE to shorten the tail.
"""

import numpy as np
import ml_dtypes

import concourse.mybir as mybir
import concourse.tile as tile
from concourse import bacc, bass_utils

B, H, W = 32, 512, 512
N_CORES = 8
ROWS_PER_CORE = B * H // N_CORES  # 2048
N_CHUNKS = ROWS_PER_CORE // 128   # 16
# m-chunks per load group; small leading groups minimize latency to the
# first matmul. 6 x-loads + 2 K-loads = 8 DMAs = the 8 HWDGE sem lanes,
# so no load queues behind another on a lane chain.
GROUP_CHUNKS = (1, 1, 2, 3, 4, 5)
N_WARM_MM = 4

# 6 column windows per chunk: (block row j, rhs col lo, rhs col hi, out col lo)
# Kept coverage per 128-col output block J is block rows {J-1, J, J+1} mod 4.
WINDOWS_BANDED = (
    (0, 384, 512, 384), (0, 0, 256, 0),
    (1, 0, 384, 0), (2, 128, 512, 128),
    (3, 256, 512, 256), (3, 0, 128, 0),
)
WINDOWS_FULL = ((0, 0, 512, 0), (1, 0, 512, 0), (2, 0, 512, 0), (3, 0, 512, 0))

_nc_cache = None
LAST_RESULTS = None  # BassKernelResults of the most recent run (for test.py)


def _build(banded: bool):
    f32 = mybir.dt.float32
    bf16 = mybir.dt.bfloat16
    wins = WINDOWS_BANDED if banded else WINDOWS_FULL

    nc = bacc.Bacc("TRN2", target_bir_lowering=False, debug=False,
                   num_devices=N_CORES)
    # x shard, transposed + group-relaid on host: for load group g,
    # xt[:, 4*go : 4*(go+gc)] holds [p, (j, c)] = x[go + c, 128*j + p].
    xt_d = nc.dram_tensor("xt", [128, 4 * ROWS_PER_CORE], bf16,
                          kind="ExternalInput").ap()
    # only the first 128 circulant rows; the other 3 row-blocks are column
    # rotations of this one (kt_j[:, q] = kt_0[:, (q - 128j) mod W]), built
    # on-device so the K DMA stays small (a single 512KB K load measured
    # ~2.5us of queue serialization ahead of the x groups)
    k_d = nc.dram_tensor("k", [128, W], bf16, kind="ExternalInput").ap()
    y_d = nc.dram_tensor("y", [ROWS_PER_CORE, W], bf16, kind="ExternalOutput").ap()

    # PE warm-up scratch: raw (untracked) SBUF, deliberately uninitialized —
    # the dummy matmul results are never read, and skipping the memset lets
    # the warm-up start as soon as the engines clear the entry barrier.
    scr = nc.alloc_sbuf_tensor("warm_scr", [128, W + 128], bf16).ap()

    group_cols = [128 * c for c in GROUP_CHUNKS]
    group_off = [128 * sum(GROUP_CHUNKS[:g]) for g in range(len(GROUP_CHUNKS))]

    with tile.TileContext(nc) as tc:
        with tc.tile_pool(name="const", bufs=1) as cpool, \
             tc.tile_pool(name="xtp", bufs=1) as xtpool, \
             tc.tile_pool(name="yout", bufs=6) as ypool, \
             tc.tile_pool(name="pyd", bufs=1, space="PSUM") as dpool, \
             tc.tile_pool(name="pyp", bufs=7, space="PSUM") as pypool:
            # Loads alternate between the two HWDGE rings (SP via nc.sync,
            # ACT via nc.scalar): a single ring's queue caps at ~130-210
            # B/ns, two active queues reach ~400. K heads the ACT ring so it
            # lands in parallel with x group 0 heading the SP ring.
            kts = [cpool.tile([128, W], bf16, name=f"kt{j}", tag=f"kt{j}")
                   for j in range(4)]
            nc.scalar.dma_start(kts[0], k_d)

            # X^T resident in SBUF as one tile per load group, loads issued
            # up-front so they head the DMA sem-lane chains.
            xtgs = []
            for g, (gc, go) in enumerate(zip(group_cols, group_off)):
                t = xtpool.tile([128, 4 * gc], bf16, name=f"xtg{g}",
                                tag=f"xtg{g}")
                eng = nc.sync if g % 2 == 0 else nc.scalar
                eng.dma_start(t, xt_d[:, 4 * go:4 * (go + gc)])
                xtgs.append(t)

            # circulant rotations, split across DVE and ACT so kt1/kt2 are
            # ready right after kt0 lands
            nc.vector.tensor_copy(kts[1][:, 128:W], kts[0][:, 0:W - 128])
            nc.vector.tensor_copy(kts[1][:, 0:128], kts[0][:, W - 128:W])
            nc.scalar.copy(kts[2][:, 256:W], kts[0][:, 0:W - 256])
            nc.scalar.copy(kts[2][:, 0:256], kts[0][:, W - 256:W])
            nc.vector.tensor_copy(kts[3][:, 384:W], kts[0][:, 0:W - 384])
            nc.vector.tensor_copy(kts[3][:, 0:384], kts[0][:, 128:W])

            # PE warm-up: dummy matmuls while the first loads are in flight.
            # HAM un-throttles the PE clock (1.2 -> 2.4 GHz) after ~3.4us of
            # sustained busy; these burn that window during DMA latency
            # instead of on real work.
            dummy = dpool.tile([128, W], f32, name="pyd", tag="pyd")
            for _ in range(N_WARM_MM):
                nc.tensor.matmul(dummy, scr[:, W:W + 128], scr[:, 0:W],
                                 start=True, stop=True)

            yo_pair = None
            for g, (nchunks, go) in enumerate(zip(GROUP_CHUNKS, group_off)):
                xtg = xtgs[g]
                gc = group_cols[g]
                for ci in range(nchunks):
                    i = go // 128 + ci
                    py = pypool.tile([128, W], f32, name=f"py{i}", tag="py")
                    for widx, (j, lo, hi, olo) in enumerate(wins):
                        lhsT = xtg[:, j * gc + 128 * ci:j * gc + 128 * (ci + 1)]
                        rhs = kts[j][:, lo:hi]
                        nc.tensor.matmul(
                            py[:, olo:olo + (hi - lo)], lhsT, rhs,
                            start=(widx == 0), stop=(widx == len(wins) - 1))
                    # chunk pairs share one [128, 1024] bf16 output tile and
                    # one 256KB store; copies (f32 PSUM -> bf16 SBUF cast)
                    # alternate DVE/ACT per pair. The last two chunks store
                    # singly so the final store is half-size; the very last
                    # chunk's copy is split across both engines.
                    copy_eng = (nc.vector.tensor_copy if (i // 2) % 2
                                else nc.scalar.copy)
                    if i >= N_CHUNKS - 2:
                        # NB: do NOT split one chunk's copy across ACT+DVE:
                        # concurrent ScalarE+VectorE access to the SAME PSUM
                        # bank is a fatal HW collision.
                        yo_s = ypool.tile([128, W], bf16, name=f"yos{i}",
                                          tag=f"yos{i % 2}", bufs=1)
                        copy_eng(yo_s, py)
                        seng = nc.sync if i % 2 == 0 else nc.scalar
                        seng.dma_start(y_d[128 * i:128 * (i + 1), :], yo_s)
                    elif i % 2 == 0:
                        yo_pair = ypool.tile([128, 2 * W], bf16,
                                             name=f"yo{i // 2}", tag="yo")
                        copy_eng(yo_pair[:, 0:W], py)
                    else:
                        copy_eng(yo_pair[:, W:2 * W], py)
                        # stores alternate rings too, behind that ring's loads
                        seng = nc.sync if (i // 2) % 2 == 0 else nc.scalar
                        seng.dma_start(
                            y_d[128 * (i - 1):128 * (i + 1), :]
                            .rearrange("(c p) q -> p c q", c=2),
                            yo_pair.rearrange("p (c q) -> p c q", c=2))

    nc.compile()
    return nc


def _filter_blocks(w: np.ndarray):
    """Circulant row-blocks (bf16) + whether banding is numerically safe."""
    taps = np.asarray(w, np.float64).reshape(-1)
    W1 = np.fft.fft(np.pad(taps, (0, W - taps.shape[0])))
    g1d = 1.0 / (np.abs(W1) ** 4)
    ker = np.fft.ifft(g1d).real
    n = np.arange(W)
    K = ker[(n[None, :] - n[:, None]) % W]  # K[n, q] = ker[(q - n) mod W]

    # banding drops blocks (bj - bi) % 4 == 2; safe when the dropped mass is
    # well under the bf16 noise floor
    drop = 0.0
    for bi in range(4):
        bj = (bi + 2) % 4
        drop += np.linalg.norm(
            K[bi * 128:(bi + 1) * 128, bj * 128:(bj + 1) * 128]) ** 2
    banded = bool(np.sqrt(drop) / np.linalg.norm(K) < 5e-3)

    kblocks = np.ascontiguousarray(
        K[0:128].astype(np.float32).astype(ml_dtypes.bfloat16))
    return kblocks, banded


def _relayout_x(xshard: np.ndarray) -> np.ndarray:
    """[2048, 512] f32 -> [128, 8192] bf16 in load-group order."""
    xt = np.ascontiguousarray(xshard.T).astype(ml_dtypes.bfloat16)
    xt4 = xt.reshape(4, 128, ROWS_PER_CORE)
    segs = []
    off = 0
    for c in GROUP_CHUNKS:
        gc = 128 * c
        segs.append(np.ascontiguousarray(
            xt4[:, :, off:off + gc].transpose(1, 0, 2).reshape(128, 4 * gc)))
        off += gc
    return np.concatenate(segs, axis=1)


def kernel(x, w) -> np.ndarray:
    global _nc_cache, LAST_RESULTS
    kblocks, banded = _filter_blocks(np.asarray(w))
    if _nc_cache is None or _nc_cache[1] != banded:
        _nc_cache = (_build(banded), banded)
    nc = _nc_cache[0]

    xf = np.asarray(x, np.float32).reshape(N_CORES, ROWS_PER_CORE, W)
    in_maps = [{"xt": _relayout_x(xf[c]), "k": kblocks}
               for c in range(N_CORES)]
    res = bass_utils.run_bass_kernel_spmd(nc, in_maps,
                                          core_ids=list(range(N_CORES)))
    LAST_RESULTS = res
    y = np.concatenate([r["y"].astype(np.float32) for r in res.results],
                       axis=0)
    return y.reshape(B, H, W, 1)
